# revision 20
# baseline (speedup 1.0000x reference)
"""Distance-scorer Bass kernel for 8 Trainium2 NeuronCores (v2).

Math: score[b,k] = W2 . relu(W1[bin,:] + x*W1[50] + y*W1[51]
                             + ego[b] @ W1[52:56] + b1) + b2
with s = x^2 + y^2, bin = clip(floor(fp32(sqrt(s)/50*50)), 0, 49).

Per core (data-parallel B-shard of 256 rows), per 8-pair chunk (16 rows):
  - s = x^2+y^2 on GPSIMD (fp32-exact).
  - One strided SBUF->SBUF DMA broadcasts each row's s across the 50
    threshold partitions (x2 element halves); a second small DMA drops the
    raw x/y values into rows 114:118 of the same tile.
  - DVE/GPSIMD tensor_scalar(is_ge, exact fp32 thresholds) overwrites the
    broadcast in place with the cumulative one-hot "staircase".
  - f32r matmul vs the differenced table -> T[bin] + x*wx + y*wy (64
    hidden, 2 elements packed per column).
  - ScalarE (+ some DVE) applies per-row ego/b1 bias + ReLU; |W2| is
    folded into the weights so a bf16 +/-1 matmul reduces to scores.
  - Scores go PSUM->DRAM directly; b2 is added on the host after gather.
"""

import numpy as np

N_CORES = 8
B, K = 2048, 1024
NB = 50
BS = B // N_CORES  # 256 rows/core
F = K // 2  # 512 moving columns; 2 elements packed per column
SELW = 114  # staircase rows: 50 A + 14 pad + 50 B
KC = 118  # mm1 contraction: 114 stair rows + 4 xy rows
CH = 8  # pairs per chunk (16 rows)
CHC = CH * K  # stair free columns per chunk (8192)
NCHUNK = BS // (2 * CH)  # 16
RPB = 64  # rows per gp/s block
DVE_COLS = 4096  # stair columns handled by DVE (rest on GPSIMD)
ACT_RELUS = 12  # relus per chunk on ScalarE (rest on DVE)


def _exact_thresholds():
    """t[j] = smallest fp32 s with floor(fp32(fp32(sqrt(s))/50)*50) >= j."""

    def bin_of(s_u32):
        s = np.uint32(s_u32).view(np.float32)
        d = np.sqrt(s, dtype=np.float32)
        v = np.float32(np.float32(d / np.float32(50.0)) * np.float32(50.0))
        return int(np.floor(v))

    ts = np.empty(NB, dtype=np.float32)
    ts[0] = -3.0e38
    for j in range(1, NB):
        lo = np.uint32(0)
        hi = np.float32(2.6e9).view(np.uint32)
        assert bin_of(hi) >= j
        while int(hi) - int(lo) > 1:
            mid = np.uint32((int(lo) + int(hi)) // 2)
            if bin_of(mid) >= j:
                hi = mid
            else:
                lo = mid
        ts[j] = np.uint32(hi).view(np.float32)
    return ts


def _consts(W1, b1, W2):
    W1 = np.asarray(W1, np.float32)
    b1 = np.asarray(b1, np.float32)
    W2 = np.asarray(W2, np.float32)
    absw2 = np.abs(W2[:, 0])
    sgn = np.where(W2[:, 0] < 0, -1.0, 1.0).astype(np.float32)

    Tt = W1[:NB] * absw2[None, :]
    dT = Tt.copy()
    dT[1:] -= Tt[:-1]
    wx = W1[50] * absw2
    wy = W1[51] * absw2

    smm1 = np.zeros((KC, 128), np.float32)
    smm1[0:50, 0:64] = dT
    smm1[64:114, 64:128] = dT
    smm1[114, 0:64] = wx
    smm1[115, 0:64] = wy
    smm1[116, 64:128] = wx
    smm1[117, 64:128] = wy

    import ml_dtypes
    bigw = np.zeros((128, 256), ml_dtypes.bfloat16)
    bigw[0:64, 126] = sgn.astype(ml_dtypes.bfloat16)
    bigw[64:128, 127] = sgn.astype(ml_dtypes.bfloat16)

    wego = np.zeros((5, 128), np.float32)
    for i in range(4):
        wego[i, 0:64] = W1[52 + i] * absw2
        wego[i, 64:128] = W1[52 + i] * absw2
    wego[4, 0:64] = b1 * absw2
    wego[4, 64:128] = b1 * absw2

    t = _exact_thresholds()
    tcol = np.full((SELW, 1), 3.0e38, np.float32)
    tcol[0:50, 0] = t
    tcol[64:114, 0] = t

    return dict(smm1=smm1, bigw=bigw, wego=wego, tcol=tcol)


def _build():
    import concourse.bass as bass
    import concourse.mybir as mybir
    from concourse import bacc
    from concourse.tile import TileContext

    f32 = mybir.dt.float32
    f32r = mybir.dt.float32r
    bf16 = mybir.dt.bfloat16
    Relu = mybir.ActivationFunctionType.Relu
    add = mybir.AluOpType.add
    mult = mybir.AluOpType.mult
    amax = mybir.AluOpType.max
    is_ge = mybir.AluOpType.is_ge

    nc = bacc.Bacc("TRN2", target_bir_lowering=False, debug=False,
                   num_devices=N_CORES)

    gpx_d = nc.declare_dram_parameter("gpx", [BS, K], f32r, isOutput=False)
    gpy_d = nc.declare_dram_parameter("gpy", [BS, K], f32r, isOutput=False)
    ego_d = nc.declare_dram_parameter("ego", [BS, 4], f32, isOutput=False)
    smm1_d = nc.declare_dram_parameter("smm1", [KC, 128], f32r, isOutput=False)
    bigw_d = nc.declare_dram_parameter("bigw", [128, 256], bf16, isOutput=False)
    wego_d = nc.declare_dram_parameter("wego", [5, 128], f32, isOutput=False)
    tcol_d = nc.declare_dram_parameter("tcol", [SELW, 1], f32, isOutput=False)
    sc_d = nc.declare_dram_parameter("scores", [BS, K], f32, isOutput=True)

    def pstride(ap_obj):
        return ap_obj.ap[0][0]

    with TileContext(nc) as tc:
        with (
            tc.tile_pool(name="consts", bufs=1) as cpool,
            tc.tile_pool(name="gp", bufs=3) as gpool,
            tc.tile_pool(name="sp", bufs=2) as spool,
            tc.tile_pool(name="stair", bufs=2) as stpool,
            tc.tile_pool(name="hr", bufs=2) as hpool,
            tc.tile_pool(name="scb", bufs=2) as scpool,
            tc.tile_pool(name="sdram", bufs=2, space="DRAM") as dpool,
            tc.tile_pool(name="p1", bufs=5, space="PSUM") as p1pool,
            tc.tile_pool(name="p2", bufs=2, space="PSUM") as p2pool,
            tc.tile_pool(name="c2pool", bufs=1, space="PSUM") as c2pool,
        ):
            smm1_s = cpool.tile([KC, 128], f32r, tag="smm1")
            bigw_s = cpool.tile([128, 256], bf16, tag="bigw")
            wego_s = cpool.tile([5, 128], f32, tag="wego")
            tcol_s = cpool.tile([SELW, 1], f32, tag="tcol")
            nc.sync.dma_start(out=smm1_s[:], in_=smm1_d[:])
            nc.sync.dma_start(out=bigw_s[:], in_=bigw_d[:])
            nc.sync.dma_start(out=wego_s[:], in_=wego_d[:])
            nc.sync.dma_start(out=tcol_s[:], in_=tcol_d[:])

            # per-row ego bias c2[:, r] = [ego[r]@W1e + b1; same] * |W2|
            ego5 = cpool.tile([5, BS], f32, tag="ego5")
            nc.vector.memset(ego5[:], 1.0)
            nc.sync.dma_start(out=ego5[0:4, :], in_=ego_d[:].rearrange("b i -> i b"))
            c2_psum = c2pool.tile([128, BS], f32, tag="c2p")
            nc.tensor.matmul(c2_psum[:], lhsT=wego_s[:], rhs=ego5[:],
                             start=True, stop=True)
            c2_s = cpool.tile([128, BS], f32, tag="c2")
            nc.scalar.copy(c2_s[:], c2_psum[:])

            NBLK = NCHUNK // 4
            gpx_t = [None] * NBLK
            gpy_t = [None] * NBLK
            s_t = [None] * NBLK
            sd_t = [None] * NBLK

            def load_block(b):
                i = b
                gpx_t[i] = gpool.tile([128, F], f32r, tag="gpx", name="gpxt")
                gpy_t[i] = gpool.tile([128, F], f32r, tag="gpy", name="gpyt")
                r0 = b * RPB
                nc.sync.dma_start(
                    out=gpx_t[i][:],
                    in_=gpx_d[r0 : r0 + RPB].rearrange("b (h k) -> (b h) k", h=2))
                nc.sync.dma_start(
                    out=gpy_t[i][:],
                    in_=gpy_d[r0 : r0 + RPB].rearrange("b (h k) -> (b h) k", h=2))

            def compute_s(b):
                i = b
                s_t[i] = spool.tile([128, F], f32r, tag="s", name="st")
                xx = spool.tile([128, F], f32r, tag="xx")
                nc.gpsimd.tensor_tensor(out=xx[:], in0=gpx_t[i][:],
                                        in1=gpx_t[i][:], op=mult)
                nc.gpsimd.tensor_tensor(out=s_t[i][:], in0=gpy_t[i][:],
                                        in1=gpy_t[i][:], op=mult)
                nc.gpsimd.tensor_tensor(out=s_t[i][:], in0=s_t[i][:],
                                        in1=xx[:], op=add)
                # stage s in DRAM: SBUF sources cannot drive the 50-way
                # partition-broadcast DMA (BIR: partition step must be dim 0)
                sd_t[i] = dpool.tile([RPB, K], f32r, tag="sd", name="sdt")
                nc.sync.dma_start(
                    out=sd_t[i][:].rearrange("b (h k) -> (b h) k", h=2),
                    in_=s_t[i][:])

            # chunk-state carried across the software pipeline
            stair_c = {}
            hr_c = {}
            p2_c = {}

            def emit_bcast(c):
                """SP: broadcast s + xy rows into the chunk's stair tile."""
                i = c // 4
                t0 = (c % 4) * 2 * CH  # row offset within the block
                st = stpool.tile([KC, CHC], f32r, tag="stair")
                stair_c[c] = st
                stp = pstride(st[:])
                sd = sd_t[i]
                # s broadcast: halves h=0/1 -> stair partitions 0:50 / 64:114
                for h, base in ((0, 0), (1, 64)):
                    src = sd[:]
                    src_b = bass.AP(
                        tensor=src.tensor,
                        offset=src.offset + t0 * K + h * F,
                        ap=[[0, 50], [K, 16], [1, F]])
                    dst = st[base : base + 50, :]
                    dst_b = bass.AP(
                        tensor=dst.tensor, offset=dst.offset,
                        ap=[[stp, 50], [1, CHC]])
                    nc.sync.dma_start(out=dst_b, in_=src_b)
                # xy rows: 114=x(h0) 115=y(h0) 116=x(h1) 117=y(h1)
                r0 = c * 2 * CH
                for gd, row in ((gpx_d, 114), (gpy_d, 115)):
                    src = gd[r0 : r0 + 2 * CH]
                    src_b = bass.AP(
                        tensor=src.tensor, offset=src.offset,
                        ap=[[F, 2], [K, 16], [1, F]])
                    dst = st[row : row + 3, :]
                    dst_b = bass.AP(
                        tensor=dst.tensor, offset=dst.offset,
                        ap=[[2 * stp, 2], [1, CHC]])
                    nc.sync.dma_start(out=dst_b, in_=src_b)

            def emit_stair(c):
                """DVE + GPSIMD: in-place cumulative staircase."""
                st = stair_c[c]
                nc.vector.tensor_scalar(
                    out=st[0:SELW, 0:DVE_COLS], in0=st[0:SELW, 0:DVE_COLS],
                    scalar1=tcol_s[:], scalar2=None, op0=is_ge)
                nc.gpsimd.tensor_scalar(
                    out=st[0:SELW, DVE_COLS:CHC], in0=st[0:SELW, DVE_COLS:CHC],
                    scalar1=tcol_s[:], scalar2=None, op0=is_ge)

            def emit_mm1_relu(c):
                """PE mm1 + Act/DVE relu into the chunk's hr tile."""
                st = stair_c[c]
                hr = hpool.tile([128, CHC], bf16, tag="hr")
                hr_c[c] = hr
                r0 = c * 2 * CH
                for p in range(CH):
                    p1a = p1pool.tile([128, F], f32, tag="p1")
                    p1b = p1pool.tile([128, F], f32, tag="p1")
                    nc.tensor.matmul(
                        p1a[:], lhsT=smm1_s[:],
                        rhs=st[0:KC, 1024 * p : 1024 * p + F],
                        start=True, stop=True)
                    nc.tensor.matmul(
                        p1b[:], lhsT=smm1_s[:],
                        rhs=st[0:KC, 1024 * p + F : 1024 * p + 1024],
                        start=True, stop=True)
                    for j, p1x in ((0, p1a), (1, p1b)):
                        r = r0 + 2 * p + j
                        dst = hr[:, 1024 * p + F * j : 1024 * p + F * (j + 1)]
                        if 2 * p + j < ACT_RELUS:
                            nc.scalar.activation(
                                dst, p1x[:], Relu,
                                bias=c2_s[:, r : r + 1], scale=1.0)
                        else:
                            nc.vector.tensor_scalar(
                                out=dst, in0=p1x[:],
                                scalar1=c2_s[:, r : r + 1], scalar2=0.0,
                                op0=add, op1=amax)

            def emit_mm2(c):
                """PE: accumulate 16 rows' scores into the superchunk bank.

                Row u of the 64-row superchunk uses the sliding window
                bigw[:, 126-2u : 254-2u]: a [128,128] lhsT that is zero
                except sgn at columns (2u, 2u+1), so out += places the
                row's scores at partitions (2u, 2u+1) of one PSUM bank.
                """
                hr = hr_c.pop(c)
                sc = c // 4
                if c % 4 == 0:
                    p2_c[sc] = p2pool.tile([128, F], f32, tag="p2", name="p2t")
                p2 = p2_c[sc]
                for t in range(2 * CH):  # 16 chunk-rows
                    u = (c % 4) * 2 * CH + t
                    nc.tensor.matmul(
                        p2[:], lhsT=bigw_s[:, 126 - 2 * u : 254 - 2 * u],
                        rhs=hr[:, 512 * t : 512 * t + F],
                        start=(u == 0), stop=(u == 63),
                        skip_group_check=True)

            def emit_scores(sc):
                """DVE copy PSUM->SBUF, then one DMA per 64 rows."""
                p2 = p2_c.pop(sc)
                scb = scpool.tile([128, F], f32, tag="scb", name="scbt")
                nc.vector.tensor_scalar_add(scb[:], p2[:], 0.0)
                r0 = sc * RPB
                dst = sc_d[r0 : r0 + RPB]
                dst_b = bass.AP(
                    tensor=dst.tensor, offset=dst.offset,
                    ap=[[K, RPB], [F, 2], [1, F]])
                nc.scalar.dma_start(out=dst_b, in_=scb[:])

            # --- software pipeline ---
            load_block(0)
            load_block(1)
            compute_s(0)
            for c in range(NCHUNK):
                if c % 4 == 1 and c // 4 + 2 < NBLK:
                    load_block(c // 4 + 2)
                if c % 4 == 2 and c // 4 + 1 < NBLK:
                    compute_s(c // 4 + 1)
                emit_bcast(c)
                emit_stair(c)
                emit_mm1_relu(c)
                if c > 0:
                    emit_mm2(c - 1)
                    if (c - 1) % 4 == 3:
                        emit_scores((c - 1) // 4)
            emit_mm2(NCHUNK - 1)
            emit_scores((NCHUNK - 1) // 4)

    nc.finalize()
    return nc


_CACHE = {}


def make_in_maps(goal_positions, ego_state, W1, b1, W2, b2):
    gp = np.asarray(goal_positions, np.float32)
    gpx = np.ascontiguousarray(gp[..., 0])
    gpy = np.ascontiguousarray(gp[..., 1])
    ego = np.ascontiguousarray(np.asarray(ego_state, np.float32))
    c = _consts(W1, b1, W2)
    in_maps = []
    for i in range(N_CORES):
        in_maps.append({
            "gpx": gpx[i * BS : (i + 1) * BS],
            "gpy": gpy[i * BS : (i + 1) * BS],
            "ego": ego[i * BS : (i + 1) * BS],
            "smm1": c["smm1"], "bigw": c["bigw"], "wego": c["wego"],
            "tcol": c["tcol"],
        })
    return in_maps


def kernel(goal_positions, ego_state, W1, b1, W2, b2):
    from concourse.bass_utils import run_bass_kernel_spmd

    if "nc" not in _CACHE:
        _CACHE["nc"] = _build()
    nc = _CACHE["nc"]

    in_maps = make_in_maps(goal_positions, ego_state, W1, b1, W2, b2)
    res = run_bass_kernel_spmd(nc, in_maps, core_ids=list(range(N_CORES)))
    out = np.concatenate([res.results[i]["scores"] for i in range(N_CORES)],
                         axis=0)
    out += np.float32(np.asarray(b2, np.float32).reshape(-1)[0])
    return out.astype(np.float32)


# revision 22
# speedup vs baseline: 3.5089x; 3.5089x over previous
"""Distance-scorer Bass kernel for 8 Trainium2 NeuronCores (v3).

Math: score[b,k] = W2 . relu(W1[bin,:] + x*W1[50] + y*W1[51]
                             + ego[b] @ W1[52:56] + b1) + b2
with d = sqrt(x^2+y^2), bin = clip(floor(d), 0, 49) (floor(d) matches the
reference fp32 binning except within ~2ulp of integer boundaries).

Per core (data-parallel B-shard of 256 rows):
  - s = x^2+y^2 (GPSIMD), d = sqrt(s) (ScalarE), bin8 = u8(d - 0.5)
    (round-nearest convert == floor) staged to DRAM as ONE BYTE per
    element: the 50-way staircase broadcast is 4x cheaper than fp32.
  - Per 16-row chunk, two DMAs broadcast bin8 across 2x50 partitions;
    the cumulative staircase is built by DVE (is_ge vs j) on even chunks
    and ScalarE (Sign(bin8-j+.5), with a dT/2 table) on odd chunks.
  - mm1 (f32r): 100 staircase rows + 4 xy rows (DMA'd from gp) + 5
    ego/bias rows (host-expanded per column) -> 64 hidden x 2 packed
    elements per column. Relu is bias-free -> one op per [128,1024].
  - mm2 (bf16): sliding +-sign window accumulates 64 rows' scores into
    one dense PSUM bank; one copy + one DMA per 64 rows.
  - b2 is added on the host after gather.
"""

import numpy as np

N_CORES = 8
B, K = 2048, 1024
NB = 50
BS = B // N_CORES  # 256 rows/core
F = K // 2  # 512 moving columns; 2 elements packed per column
SELW = 100  # staircase rows: 50 per element half
KC = 109  # mm1 contraction: 100 stair + 4 xy + 5 ego/bias rows
CH = 8  # pairs per chunk (16 rows)
CHC = CH * K  # stair free columns per chunk (8192)
NCHUNK = BS // (2 * CH)  # 16
RPB = 64  # rows per gp/s block
BSF = BS * F  # egob columns


def _consts(W1, b1, W2):
    import ml_dtypes

    W1 = np.asarray(W1, np.float32)
    b1 = np.asarray(b1, np.float32)
    W2 = np.asarray(W2, np.float32)
    absw2 = np.abs(W2[:, 0])
    sgn = np.where(W2[:, 0] < 0, -1.0, 1.0).astype(np.float32)

    Tt = W1[:NB] * absw2[None, :]
    dT = Tt.copy()
    dT[1:] -= Tt[:-1]
    wx = W1[50] * absw2
    wy = W1[51] * absw2

    def mk_smm1(table, ones_w):
        m = np.zeros((KC, 128), np.float32)
        m[0:50, 0:64] = table
        m[50:100, 64:128] = table
        m[100, 0:64] = wx
        m[101, 0:64] = wy
        m[102, 64:128] = wx
        m[103, 64:128] = wy
        for i in range(4):
            m[104 + i, 0:64] = W1[52 + i] * absw2
            m[104 + i, 64:128] = W1[52 + i] * absw2
        m[108, 0:64] = ones_w
        m[108, 64:128] = ones_w
        return m

    b1w = b1 * absw2
    smm1a = mk_smm1(dT, b1w)  # 0/1 staircase (DVE is_ge)
    smm1b = mk_smm1(dT / 2, b1w + Tt[NB - 1] / 2)  # +-1 staircase (Sign)

    bigw = np.zeros((128, 256), ml_dtypes.bfloat16)
    bigw[0:64, 126] = sgn.astype(ml_dtypes.bfloat16)
    bigw[64:128, 127] = sgn.astype(ml_dtypes.bfloat16)

    j = np.arange(NB, dtype=np.float32)
    tcolf = np.concatenate([j, j]).reshape(SELW, 1)  # is_ge thresholds
    tsign = -(np.concatenate([j, j]) - 0.5).reshape(SELW, 1)  # Sign bias

    return dict(smm1a=smm1a, smm1b=smm1b, bigw=bigw, tcolf=tcolf,
                tsign=tsign)


def _build():
    import concourse.bass as bass
    import concourse.mybir as mybir
    from concourse import bacc
    from concourse.tile import TileContext

    f32 = mybir.dt.float32
    f32r = mybir.dt.float32r
    bf16 = mybir.dt.bfloat16
    u8 = mybir.dt.uint8
    Relu = mybir.ActivationFunctionType.Relu
    Sign = mybir.ActivationFunctionType.Sign
    Sqrt = mybir.ActivationFunctionType.Sqrt
    Copy = mybir.ActivationFunctionType.Copy
    add = mybir.AluOpType.add
    mult = mybir.AluOpType.mult
    amax = mybir.AluOpType.max
    is_ge = mybir.AluOpType.is_ge

    nc = bacc.Bacc("TRN2", target_bir_lowering=False, debug=False,
                   num_devices=N_CORES)

    gpx_d = nc.declare_dram_parameter("gpx", [BS, K], f32r, isOutput=False)
    gpy_d = nc.declare_dram_parameter("gpy", [BS, K], f32r, isOutput=False)
    egob_d = nc.declare_dram_parameter("egob", [5, BSF], f32r, isOutput=False)
    smm1a_d = nc.declare_dram_parameter("smm1a", [KC, 128], f32r,
                                        isOutput=False)
    smm1b_d = nc.declare_dram_parameter("smm1b", [KC, 128], f32r,
                                        isOutput=False)
    bigw_d = nc.declare_dram_parameter("bigw", [128, 256], bf16,
                                       isOutput=False)
    tcolf_d = nc.declare_dram_parameter("tcolf", [SELW, 1], f32,
                                        isOutput=False)
    tsign_d = nc.declare_dram_parameter("tsign", [SELW, 1], f32,
                                        isOutput=False)
    sc_d = nc.declare_dram_parameter("scores", [BS, K], f32, isOutput=True)

    def pstride(ap_obj):
        return ap_obj.ap[0][0]

    with TileContext(nc) as tc:
        with (
            tc.tile_pool(name="consts", bufs=1) as cpool,
            tc.tile_pool(name="gp", bufs=3) as gpool,
            tc.tile_pool(name="sp", bufs=2) as spool,
            tc.tile_pool(name="mp8", bufs=2) as mpool,
            tc.tile_pool(name="stair", bufs=2) as stpool,
            tc.tile_pool(name="hr", bufs=2) as hpool,
            tc.tile_pool(name="scb", bufs=2) as scpool,
            tc.tile_pool(name="bdram", bufs=2, space="DRAM") as dpool,
            tc.tile_pool(name="p1", bufs=2, space="PSUM") as p1pool,
            tc.tile_pool(name="p2", bufs=2, space="PSUM") as p2pool,
        ):
            smm1a_s = cpool.tile([KC, 128], f32r, tag="smm1a")
            smm1b_s = cpool.tile([KC, 128], f32r, tag="smm1b")
            bigw_s = cpool.tile([128, 256], bf16, tag="bigw")
            tcolf_s = cpool.tile([SELW, 1], f32, tag="tcolf")
            tsign_s = cpool.tile([SELW, 1], f32, tag="tsign")
            nc.sync.dma_start(out=smm1a_s[:], in_=smm1a_d[:])
            nc.sync.dma_start(out=smm1b_s[:], in_=smm1b_d[:])
            nc.sync.dma_start(out=bigw_s[:], in_=bigw_d[:])
            nc.sync.dma_start(out=tcolf_s[:], in_=tcolf_d[:])
            nc.sync.dma_start(out=tsign_s[:], in_=tsign_d[:])

            NBLK = NCHUNK // 4
            gpx_t = [None] * NBLK
            gpy_t = [None] * NBLK
            b8_t = [None] * NBLK

            def load_block(b):
                i = b
                gpx_t[i] = gpool.tile([128, F], f32r, tag="gpx", name="gpxt")
                gpy_t[i] = gpool.tile([128, F], f32r, tag="gpy", name="gpyt")
                r0 = b * RPB
                nc.sync.dma_start(
                    out=gpx_t[i][:],
                    in_=gpx_d[r0 : r0 + RPB].rearrange("b (h k) -> (b h) k",
                                                       h=2))
                nc.sync.dma_start(
                    out=gpy_t[i][:],
                    in_=gpy_d[r0 : r0 + RPB].rearrange("b (h k) -> (b h) k",
                                                       h=2))

            def compute_bin8(b):
                i = b
                s_pair = spool.tile([128, F], f32r, tag="s", name="spt")
                xx = spool.tile([128, F], f32r, tag="xx")
                nc.gpsimd.tensor_tensor(out=xx[:], in0=gpx_t[i][:],
                                        in1=gpx_t[i][:], op=mult)
                nc.gpsimd.tensor_tensor(out=s_pair[:], in0=gpy_t[i][:],
                                        in1=gpy_t[i][:], op=mult)
                nc.gpsimd.tensor_tensor(out=s_pair[:], in0=s_pair[:],
                                        in1=xx[:], op=add)
                dtile = spool.tile([128, F], f32, tag="d", name="dt")
                nc.scalar.activation(dtile[:], s_pair[:], Sqrt)
                bin8 = spool.tile([128, F], u8, tag="b8", name="b8t")
                # round-nearest u8 convert of max(d - 0.5, 0) == floor(d)
                nc.vector.tensor_scalar(out=bin8[:], in0=dtile[:],
                                        scalar1=-0.5, scalar2=0.0, op0=add,
                                        op1=amax)
                b8_t[i] = dpool.tile([RPB, K], u8, tag="b8d", name="b8d")
                nc.sync.dma_start(
                    out=b8_t[i][:].rearrange("b (h k) -> (b h) k", h=2),
                    in_=bin8[:])

            # chunk-state carried across the software pipeline
            stair_c = {}
            hr_c = {}
            p2_c = {}

            def emit_bcast(c):
                """SP: bin8 broadcast + xy + ego rows for chunk c."""
                i = c // 4
                t0 = (c % 4) * 2 * CH  # row offset within the block
                mp8 = mpool.tile([SELW, CHC], u8, tag="mp8", name="mp8t")
                st = stpool.tile([KC, CHC], f32r, tag="stair", name="stairt")
                stair_c[c] = (mp8, st)
                stp = pstride(st[:])
                mpp = pstride(mp8[:])
                bd = b8_t[i]
                # bin8 broadcast: halves h=0/1 -> mp8 partitions 0:50 / 50:100
                for h in (0, 1):
                    src = bd[:]
                    src_b = bass.AP(
                        tensor=src.tensor,
                        offset=src.offset + t0 * K + h * F,
                        ap=[[0, 50], [K, 16], [1, F]])
                    dst = mp8[50 * h : 50 * h + 50, :]
                    dst_b = bass.AP(
                        tensor=dst.tensor, offset=dst.offset,
                        ap=[[mpp, 50], [1, CHC]])
                    nc.sync.dma_start(out=dst_b, in_=src_b)
                # xy rows: 100=x(h0) 101=y(h0) 102=x(h1) 103=y(h1)
                r0 = c * 2 * CH
                for gd, row in ((gpx_d, 100), (gpy_d, 101)):
                    src = gd[r0 : r0 + 2 * CH]
                    src_b = bass.AP(
                        tensor=src.tensor, offset=src.offset,
                        ap=[[F, 2], [K, 16], [1, F]])
                    dst = st[row : row + 3, :]
                    dst_b = bass.AP(
                        tensor=dst.tensor, offset=dst.offset,
                        ap=[[2 * stp, 2], [1, CHC]])
                    nc.sync.dma_start(out=dst_b, in_=src_b)
                # ego/bias rows 104:109 (host-expanded per column)
                src = egob_d[:]
                src_b = bass.AP(
                    tensor=src.tensor, offset=src.offset + c * CHC,
                    ap=[[BSF, 5], [1, CHC]])
                dst = st[104:109, :]
                dst_b = bass.AP(
                    tensor=dst.tensor, offset=dst.offset,
                    ap=[[stp, 5], [1, CHC]])
                nc.sync.dma_start(out=dst_b, in_=src_b)

            def emit_stair(c):
                """Cumulative staircase: DVE (0/1) even chunks, Act (+-1)
                odd chunks."""
                mp8, st = stair_c[c]
                if c % 2 == 0:
                    nc.vector.tensor_scalar(
                        out=st[0:SELW, :], in0=mp8[:],
                        scalar1=tcolf_s[:], scalar2=None, op0=is_ge)
                else:
                    nc.scalar.activation(
                        st[0:SELW, :], mp8[:], Sign,
                        bias=tsign_s[:, 0:1], scale=1.0)

            def emit_mm1_relu(c):
                """PE mm1 + bias-free relu (DVE/Act rotation) into hr."""
                _, st = stair_c[c]
                lhs = smm1a_s if c % 2 == 0 else smm1b_s
                hr = hpool.tile([128, CHC], bf16, tag="hr", name="hrt")
                hr_c[c] = hr
                # relu engine per pair: on DVE-stair chunks Act takes more
                dve_pairs = 2 if c % 2 == 0 else 7
                for p in range(CH):
                    p1s = p1pool.tile([128, 2 * F], f32, tag="p1",
                                      name="p1t")
                    nc.tensor.matmul(
                        p1s[:, 0:F], lhsT=lhs[:],
                        rhs=st[0:KC, 1024 * p : 1024 * p + F],
                        start=True, stop=True)
                    nc.tensor.matmul(
                        p1s[:, F : 2 * F], lhsT=lhs[:],
                        rhs=st[0:KC, 1024 * p + F : 1024 * p + 1024],
                        start=True, stop=True)
                    dst = hr[:, 1024 * p : 1024 * p + 1024]
                    if p < dve_pairs:
                        nc.vector.tensor_scalar(
                            out=dst, in0=p1s[:], scalar1=0.0, scalar2=None,
                            op0=amax)
                    else:
                        nc.scalar.activation(dst, p1s[:], Relu)

            def emit_mm2(c):
                """PE: accumulate 16 rows' scores into the superchunk bank
                via the sliding +-sign window."""
                hr = hr_c.pop(c)
                sc = c // 4
                if c % 4 == 0:
                    p2_c[sc] = p2pool.tile([128, F], f32, tag="p2",
                                           name="p2t")
                p2 = p2_c[sc]
                for t in range(2 * CH):
                    u = (c % 4) * 2 * CH + t
                    nc.tensor.matmul(
                        p2[:], lhsT=bigw_s[:, 126 - 2 * u : 254 - 2 * u],
                        rhs=hr[:, 512 * t : 512 * t + F],
                        start=(u == 0), stop=(u == 63),
                        skip_group_check=True)

            def emit_scores(sc):
                """DVE copy PSUM->SBUF, then one DMA per 64 rows."""
                p2 = p2_c.pop(sc)
                scb = scpool.tile([128, F], f32, tag="scb", name="scbt")
                nc.vector.tensor_scalar_add(scb[:], p2[:], 0.0)
                r0 = sc * RPB
                dst = sc_d[r0 : r0 + RPB]
                dst_b = bass.AP(
                    tensor=dst.tensor, offset=dst.offset,
                    ap=[[K, RPB], [F, 2], [1, F]])
                nc.scalar.dma_start(out=dst_b, in_=scb[:])

            # --- software pipeline ---
            load_block(0)
            load_block(1)
            compute_bin8(0)
            for c in range(NCHUNK):
                if c % 4 == 1 and c // 4 + 2 < NBLK:
                    load_block(c // 4 + 2)
                if c % 4 == 2 and c // 4 + 1 < NBLK:
                    compute_bin8(c // 4 + 1)
                emit_bcast(c)
                emit_stair(c)
                if c > 0:
                    emit_mm2(c - 1)
                emit_mm1_relu(c)
                if c > 0 and (c - 1) % 4 == 3:
                    emit_scores((c - 1) // 4)
            emit_mm2(NCHUNK - 1)
            emit_scores((NCHUNK - 1) // 4)

    nc.finalize()
    return nc


_CACHE = {}


def make_in_maps(goal_positions, ego_state, W1, b1, W2, b2):
    gp = np.asarray(goal_positions, np.float32)
    gpx = np.ascontiguousarray(gp[..., 0])
    gpy = np.ascontiguousarray(gp[..., 1])
    ego = np.asarray(ego_state, np.float32)
    c = _consts(W1, b1, W2)
    in_maps = []
    for i in range(N_CORES):
        eg = ego[i * BS : (i + 1) * BS]  # [BS, 4]
        egob = np.empty((5, BS, F), np.float32)
        egob[0:4] = np.repeat(eg.T[:, :, None], F, axis=2)
        egob[4] = 1.0
        in_maps.append({
            "gpx": gpx[i * BS : (i + 1) * BS],
            "gpy": gpy[i * BS : (i + 1) * BS],
            "egob": egob.reshape(5, BSF),
            "smm1a": c["smm1a"], "smm1b": c["smm1b"], "bigw": c["bigw"],
            "tcolf": c["tcolf"], "tsign": c["tsign"],
        })
    return in_maps


def kernel(goal_positions, ego_state, W1, b1, W2, b2):
    from concourse.bass_utils import run_bass_kernel_spmd

    if "nc" not in _CACHE:
        _CACHE["nc"] = _build()
    nc = _CACHE["nc"]

    in_maps = make_in_maps(goal_positions, ego_state, W1, b1, W2, b2)
    res = run_bass_kernel_spmd(nc, in_maps, core_ids=list(range(N_CORES)))
    out = np.concatenate([res.results[i]["scores"] for i in range(N_CORES)],
                         axis=0)
    out += np.float32(np.asarray(b2, np.float32).reshape(-1)[0])
    return out.astype(np.float32)


# revision 23
# speedup vs baseline: 4.4935x; 1.2806x over previous
"""Distance-scorer Bass kernel for 8 Trainium2 NeuronCores (v3).

Math: score[b,k] = W2 . relu(W1[bin,:] + x*W1[50] + y*W1[51]
                             + ego[b] @ W1[52:56] + b1) + b2
with d = sqrt(x^2+y^2), bin = clip(floor(d), 0, 49) (floor(d) matches the
reference fp32 binning except within ~2ulp of integer boundaries).

Per core (data-parallel B-shard of 256 rows):
  - s = x^2+y^2 (GPSIMD), d = sqrt(s) (ScalarE), bin8 = u8(d - 0.5)
    (round-nearest convert == floor) staged to DRAM as ONE BYTE per
    element: the 50-way staircase broadcast is 4x cheaper than fp32.
  - Per 16-row chunk, two DMAs broadcast bin8 across 2x50 partitions;
    the cumulative staircase is built by DVE (is_ge vs j) on even chunks
    and ScalarE (Sign(bin8-j+.5), with a dT/2 table) on odd chunks.
  - mm1 (f32r): 100 staircase rows + 4 xy rows (DMA'd from gp) + 5
    ego/bias rows (host-expanded per column) -> 64 hidden x 2 packed
    elements per column. Relu is bias-free -> one op per [128,1024].
  - mm2 (bf16): sliding +-sign window accumulates 64 rows' scores into
    one dense PSUM bank; one copy + one DMA per 64 rows.
  - b2 is added on the host after gather.
"""

import numpy as np

N_CORES = 8
B, K = 2048, 1024
NB = 50
BS = B // N_CORES  # 256 rows/core
F = K // 2  # 512 moving columns; 2 elements packed per column
SELW = 100  # staircase rows: 50 per element half
KC = 109  # mm1 contraction: 100 stair + 4 xy + 5 ego/bias rows
CH = 8  # pairs per chunk (16 rows)
CHC = CH * K  # stair free columns per chunk (8192)
NCHUNK = BS // (2 * CH)  # 16
RPB = 64  # rows per gp/s block
BSF = BS * F  # egob columns


def _consts(W1, b1, W2):
    import ml_dtypes

    W1 = np.asarray(W1, np.float32)
    b1 = np.asarray(b1, np.float32)
    W2 = np.asarray(W2, np.float32)
    absw2 = np.abs(W2[:, 0])
    sgn = np.where(W2[:, 0] < 0, -1.0, 1.0).astype(np.float32)

    Tt = W1[:NB] * absw2[None, :]
    dT = Tt.copy()
    dT[1:] -= Tt[:-1]
    wx = W1[50] * absw2
    wy = W1[51] * absw2

    def mk_smm1(table, ones_w):
        m = np.zeros((KC, 128), np.float32)
        m[0:50, 0:64] = table
        m[50:100, 64:128] = table
        m[100, 0:64] = wx
        m[101, 0:64] = wy
        m[102, 64:128] = wx
        m[103, 64:128] = wy
        for i in range(4):
            m[104 + i, 0:64] = W1[52 + i] * absw2
            m[104 + i, 64:128] = W1[52 + i] * absw2
        m[108, 0:64] = ones_w
        m[108, 64:128] = ones_w
        return m

    b1w = b1 * absw2
    smm1a = mk_smm1(dT, b1w)  # 0/1 staircase (DVE is_ge)
    smm1b = mk_smm1(dT / 2, b1w + Tt[NB - 1] / 2)  # +-1 staircase (Sign)

    bigw = np.zeros((128, 256), ml_dtypes.bfloat16)
    bigw[0:64, 126] = sgn.astype(ml_dtypes.bfloat16)
    bigw[64:128, 127] = sgn.astype(ml_dtypes.bfloat16)

    j = np.arange(NB, dtype=np.float32)
    tcolf = np.concatenate([j, j]).reshape(SELW, 1)  # is_ge thresholds
    tsign = -(np.concatenate([j, j]) - 0.5).reshape(SELW, 1)  # Sign bias

    return dict(smm1a=smm1a, smm1b=smm1b, bigw=bigw, tcolf=tcolf,
                tsign=tsign)


def _build():
    import concourse.bass as bass
    import concourse.mybir as mybir
    from concourse import bacc
    from concourse.tile import TileContext

    f32 = mybir.dt.float32
    f32r = mybir.dt.float32r
    bf16 = mybir.dt.bfloat16
    u8 = mybir.dt.uint8
    Relu = mybir.ActivationFunctionType.Relu
    Sign = mybir.ActivationFunctionType.Sign
    Sqrt = mybir.ActivationFunctionType.Sqrt
    Copy = mybir.ActivationFunctionType.Copy
    add = mybir.AluOpType.add
    mult = mybir.AluOpType.mult
    amax = mybir.AluOpType.max
    is_ge = mybir.AluOpType.is_ge

    nc = bacc.Bacc("TRN2", target_bir_lowering=False, debug=False,
                   num_devices=N_CORES)

    gpx_d = nc.declare_dram_parameter("gpx", [BS, K], f32r, isOutput=False)
    gpy_d = nc.declare_dram_parameter("gpy", [BS, K], f32r, isOutput=False)
    egob_d = nc.declare_dram_parameter("egob", [5, BSF], f32r, isOutput=False)
    smm1a_d = nc.declare_dram_parameter("smm1a", [KC, 128], f32r,
                                        isOutput=False)
    smm1b_d = nc.declare_dram_parameter("smm1b", [KC, 128], f32r,
                                        isOutput=False)
    bigw_d = nc.declare_dram_parameter("bigw", [128, 256], bf16,
                                       isOutput=False)
    tcolf_d = nc.declare_dram_parameter("tcolf", [SELW, 1], f32,
                                        isOutput=False)
    tsign_d = nc.declare_dram_parameter("tsign", [SELW, 1], f32,
                                        isOutput=False)
    sc_d = nc.declare_dram_parameter("scores", [BS, K], f32, isOutput=True)

    def pstride(ap_obj):
        return ap_obj.ap[0][0]

    with TileContext(nc) as tc:
        with (
            tc.tile_pool(name="consts", bufs=1) as cpool,
            tc.tile_pool(name="gp", bufs=3) as gpool,
            tc.tile_pool(name="sp", bufs=2) as spool,
            tc.tile_pool(name="mp8", bufs=3) as mpool,
            tc.tile_pool(name="stair", bufs=2) as stpool,
            tc.tile_pool(name="hr", bufs=2) as hpool,
            tc.tile_pool(name="scb", bufs=2) as scpool,
            tc.tile_pool(name="bdram", bufs=2, space="DRAM") as dpool,
            tc.tile_pool(name="p1", bufs=2, space="PSUM") as p1pool,
            tc.tile_pool(name="p2", bufs=2, space="PSUM") as p2pool,
        ):
            smm1a_s = cpool.tile([KC, 128], f32r, tag="smm1a")
            smm1b_s = cpool.tile([KC, 128], f32r, tag="smm1b")
            bigw_s = cpool.tile([128, 256], bf16, tag="bigw")
            tcolf_s = cpool.tile([SELW, 1], f32, tag="tcolf")
            tsign_s = cpool.tile([SELW, 1], f32, tag="tsign")
            nc.sync.dma_start(out=smm1a_s[:], in_=smm1a_d[:])
            nc.sync.dma_start(out=smm1b_s[:], in_=smm1b_d[:])
            nc.sync.dma_start(out=bigw_s[:], in_=bigw_d[:])
            nc.sync.dma_start(out=tcolf_s[:], in_=tcolf_d[:])
            nc.sync.dma_start(out=tsign_s[:], in_=tsign_d[:])

            NBLK = NCHUNK // 4
            gpx_t = [None] * NBLK
            gpy_t = [None] * NBLK
            b8_t = [None] * NBLK

            def load_block(b):
                i = b
                gpx_t[i] = gpool.tile([128, F], f32r, tag="gpx", name="gpxt")
                gpy_t[i] = gpool.tile([128, F], f32r, tag="gpy", name="gpyt")
                r0 = b * RPB
                nc.sync.dma_start(
                    out=gpx_t[i][:],
                    in_=gpx_d[r0 : r0 + RPB].rearrange("b (h k) -> (b h) k",
                                                       h=2))
                nc.sync.dma_start(
                    out=gpy_t[i][:],
                    in_=gpy_d[r0 : r0 + RPB].rearrange("b (h k) -> (b h) k",
                                                       h=2))

            def compute_bin8(b):
                i = b
                s_pair = spool.tile([128, F], f32r, tag="s", name="spt")
                xx = spool.tile([128, F], f32r, tag="xx")
                nc.gpsimd.tensor_tensor(out=xx[:], in0=gpx_t[i][:],
                                        in1=gpx_t[i][:], op=mult)
                nc.gpsimd.tensor_tensor(out=s_pair[:], in0=gpy_t[i][:],
                                        in1=gpy_t[i][:], op=mult)
                nc.gpsimd.tensor_tensor(out=s_pair[:], in0=s_pair[:],
                                        in1=xx[:], op=add)
                dtile = spool.tile([128, F], f32, tag="d", name="dt")
                nc.scalar.activation(dtile[:], s_pair[:], Sqrt)
                bin8 = spool.tile([128, F], u8, tag="b8", name="b8t")
                # round-nearest u8 convert of max(d - 0.5, 0) == floor(d)
                nc.vector.tensor_scalar(out=bin8[:], in0=dtile[:],
                                        scalar1=-0.5, scalar2=0.0, op0=add,
                                        op1=amax)
                b8_t[i] = dpool.tile([RPB, K], u8, tag="b8d", name="b8d")
                nc.sync.dma_start(
                    out=b8_t[i][:].rearrange("b (h k) -> (b h) k", h=2),
                    in_=bin8[:])

            # chunk-state carried across the software pipeline
            stair_c = {}
            hr_c = {}
            p2_c = {}

            def emit_bcast(c):
                """SP: bin8 broadcast + xy + ego rows for chunk c."""
                i = c // 4
                t0 = (c % 4) * 2 * CH  # row offset within the block
                mp8 = mpool.tile([SELW, CHC], u8, tag="mp8", name="mp8t")
                st = stpool.tile([KC, CHC], f32r, tag="stair", name="stairt")
                stair_c[c] = (mp8, st)
                stp = pstride(st[:])
                mpp = pstride(mp8[:])
                bd = b8_t[i]
                # bin8 broadcast: halves h=0/1 -> mp8 partitions 0:50 / 50:100
                for h in (0, 1):
                    src = bd[:]
                    src_b = bass.AP(
                        tensor=src.tensor,
                        offset=src.offset + t0 * K + h * F,
                        ap=[[0, 50], [K, 16], [1, F]])
                    dst = mp8[50 * h : 50 * h + 50, :]
                    dst_b = bass.AP(
                        tensor=dst.tensor, offset=dst.offset,
                        ap=[[mpp, 50], [1, CHC]])
                    nc.sync.dma_start(out=dst_b, in_=src_b)
                # xy rows: 100=x(h0) 101=y(h0) 102=x(h1) 103=y(h1)
                r0 = c * 2 * CH
                for gd, row in ((gpx_d, 100), (gpy_d, 101)):
                    src = gd[r0 : r0 + 2 * CH]
                    src_b = bass.AP(
                        tensor=src.tensor, offset=src.offset,
                        ap=[[F, 2], [K, 16], [1, F]])
                    dst = st[row : row + 3, :]
                    dst_b = bass.AP(
                        tensor=dst.tensor, offset=dst.offset,
                        ap=[[2 * stp, 2], [1, CHC]])
                    nc.sync.dma_start(out=dst_b, in_=src_b)
                # ego/bias rows 104:109 (host-expanded per column)
                src = egob_d[:]
                src_b = bass.AP(
                    tensor=src.tensor, offset=src.offset + c * CHC,
                    ap=[[BSF, 5], [1, CHC]])
                dst = st[104:109, :]
                dst_b = bass.AP(
                    tensor=dst.tensor, offset=dst.offset,
                    ap=[[stp, 5], [1, CHC]])
                nc.sync.dma_start(out=dst_b, in_=src_b)

            def emit_stair(c):
                """Cumulative staircase: DVE (0/1) even chunks, Act (+-1)
                odd chunks."""
                mp8, st = stair_c[c]
                if c % 2 == 0:
                    nc.vector.tensor_scalar(
                        out=st[0:SELW, :], in0=mp8[:],
                        scalar1=tcolf_s[:], scalar2=None, op0=is_ge)
                else:
                    nc.scalar.activation(
                        st[0:SELW, :], mp8[:], Sign,
                        bias=tsign_s[:, 0:1], scale=1.0)

            def emit_mm1_relu(c):
                """PE mm1 + bias-free relu (DVE/Act rotation) into hr."""
                _, st = stair_c[c]
                lhs = smm1a_s if c % 2 == 0 else smm1b_s
                hr = hpool.tile([128, CHC], bf16, tag="hr", name="hrt")
                hr_c[c] = hr
                # relus go mostly to the engine NOT building stair(c+1)
                if c + 1 < NCHUNK:
                    dve_pairs = 1 if (c + 1) % 2 == 0 else 7
                else:
                    dve_pairs = 4
                for p in range(CH):
                    p1s = p1pool.tile([128, 2 * F], f32, tag="p1",
                                      name="p1t")
                    nc.tensor.matmul(
                        p1s[:, 0:F], lhsT=lhs[:],
                        rhs=st[0:KC, 1024 * p : 1024 * p + F],
                        start=True, stop=True)
                    nc.tensor.matmul(
                        p1s[:, F : 2 * F], lhsT=lhs[:],
                        rhs=st[0:KC, 1024 * p + F : 1024 * p + 1024],
                        start=True, stop=True)
                    dst = hr[:, 1024 * p : 1024 * p + 1024]
                    if p < dve_pairs:
                        nc.vector.tensor_scalar(
                            out=dst, in0=p1s[:], scalar1=0.0, scalar2=None,
                            op0=amax)
                    else:
                        nc.scalar.activation(dst, p1s[:], Relu)

            def emit_mm2(c):
                """PE: accumulate 16 rows' scores into the superchunk bank
                via the sliding +-sign window."""
                hr = hr_c.pop(c)
                sc = c // 4
                if c % 4 == 0:
                    p2_c[sc] = p2pool.tile([128, F], f32, tag="p2",
                                           name="p2t")
                p2 = p2_c[sc]
                for t in range(2 * CH):
                    u = (c % 4) * 2 * CH + t
                    nc.tensor.matmul(
                        p2[:], lhsT=bigw_s[:, 126 - 2 * u : 254 - 2 * u],
                        rhs=hr[:, 512 * t : 512 * t + F],
                        start=(u == 0), stop=(u == 63),
                        skip_group_check=True)

            def emit_scores(sc):
                """DVE copy PSUM->SBUF, then one DMA per 64 rows."""
                p2 = p2_c.pop(sc)
                scb = scpool.tile([128, F], f32, tag="scb", name="scbt")
                nc.vector.tensor_scalar_add(scb[:], p2[:], 0.0)
                r0 = sc * RPB
                dst = sc_d[r0 : r0 + RPB]
                dst_b = bass.AP(
                    tensor=dst.tensor, offset=dst.offset,
                    ap=[[K, RPB], [F, 2], [1, F]])
                nc.scalar.dma_start(out=dst_b, in_=scb[:])

            # --- software pipeline (stair built one chunk ahead) ---
            load_block(0)
            load_block(1)
            compute_bin8(0)
            emit_bcast(0)
            emit_stair(0)
            emit_bcast(1)
            for c in range(NCHUNK):
                if c % 4 == 1 and c // 4 + 2 < NBLK:
                    load_block(c // 4 + 2)
                if c % 4 == 2 and c // 4 + 1 < NBLK:
                    compute_bin8(c // 4 + 1)
                if c + 2 < NCHUNK:
                    emit_bcast(c + 2)
                if c + 1 < NCHUNK:
                    emit_stair(c + 1)
                if c > 0:
                    emit_mm2(c - 1)
                emit_mm1_relu(c)
                if c > 0 and (c - 1) % 4 == 3:
                    emit_scores((c - 1) // 4)
            emit_mm2(NCHUNK - 1)
            emit_scores((NCHUNK - 1) // 4)

    nc.finalize()
    return nc


_CACHE = {}


def make_in_maps(goal_positions, ego_state, W1, b1, W2, b2):
    gp = np.asarray(goal_positions, np.float32)
    gpx = np.ascontiguousarray(gp[..., 0])
    gpy = np.ascontiguousarray(gp[..., 1])
    ego = np.asarray(ego_state, np.float32)
    c = _consts(W1, b1, W2)
    in_maps = []
    for i in range(N_CORES):
        eg = ego[i * BS : (i + 1) * BS]  # [BS, 4]
        egob = np.empty((5, BS, F), np.float32)
        egob[0:4] = np.repeat(eg.T[:, :, None], F, axis=2)
        egob[4] = 1.0
        in_maps.append({
            "gpx": gpx[i * BS : (i + 1) * BS],
            "gpy": gpy[i * BS : (i + 1) * BS],
            "egob": egob.reshape(5, BSF),
            "smm1a": c["smm1a"], "smm1b": c["smm1b"], "bigw": c["bigw"],
            "tcolf": c["tcolf"], "tsign": c["tsign"],
        })
    return in_maps


def kernel(goal_positions, ego_state, W1, b1, W2, b2):
    from concourse.bass_utils import run_bass_kernel_spmd

    if "nc" not in _CACHE:
        _CACHE["nc"] = _build()
    nc = _CACHE["nc"]

    in_maps = make_in_maps(goal_positions, ego_state, W1, b1, W2, b2)
    res = run_bass_kernel_spmd(nc, in_maps, core_ids=list(range(N_CORES)))
    out = np.concatenate([res.results[i]["scores"] for i in range(N_CORES)],
                         axis=0)
    out += np.float32(np.asarray(b2, np.float32).reshape(-1)[0])
    return out.astype(np.float32)
